# revision 4
# baseline (speedup 1.0000x reference)
"""DenoiseGCN Trainium2 kernel.

Full-input contract: kernel(**inputs) takes the unsharded inputs from
setup_inputs() and returns the full (512, 2048) float32 output.

Strategy: pure data parallel over 8 NeuronCores (64 samples each, no
collectives). Activations stay resident in SBUF in a feature-major
layout ([features -> partitions, vertices -> free dim]).

v10: pair-granular pipeline (2 samples per tile) on top of v9:
  * PSUM tiles are [128, 2*1024] fp32 (4 banks) holding BOTH samples of
    a pair for one m-chunk; ONE [128, 2048] ACTIVATE per (pair, layer,
    m) with the shared per-m bias column - scalar-engine instruction
    count halves and the 222-cycle init amortizes over 2048 elems.
  * h/g/hs tiles hold the pair ([128, 4, *] with j = s*2+m), so each
    DVE neighbor-sum TENSOR_TENSOR covers 4096 elems at 2x.
  * matmul loops are k-outer so consecutive matmuls share the
    stationary operand (fewer LDWEIGHTS in the PE stream).
  * l0 keeps per-sample acts (its bias is per-sample), reading halves
    of the wide PSUM tile.
"""

import numpy as np
import ml_dtypes

import concourse.bacc as bacc
import concourse.mybir as mybir
import concourse.tile as tile
from concourse.bass_utils import run_bass_kernel_spmd

F32 = mybir.dt.float32
F32R = mybir.dt.float32r
BF16 = mybir.dt.bfloat16
AF = mybir.ActivationFunctionType
ALU = mybir.AluOpType

NCORES = 8
B = 512
BPC = B // NCORES  # samples per core
V = 1024           # vertices per sample
HID = 256
TDIM = 128
DATA = 2048
HW = 1026          # haloed row width per feature-tile (1 + 1024 + 1)

def _sin_table():
    half = TDIM // 2
    freqs = np.exp(-np.log(10000.0) * np.arange(half, dtype=np.float64) / (half - 1))
    tt = np.arange(1000, dtype=np.float64)[:, None] * freqs[None, :]
    return np.concatenate([np.sin(tt), np.cos(tt)], axis=1).astype(np.float32)


_SIN_TABLE = _sin_table()

_PROG = None


def _build():
    nc = bacc.Bacc("TRN2", target_bir_lowering=False, debug=False, num_devices=NCORES)

    x = nc.dram_tensor("x", [2 * BPC, V], F32, kind="ExternalInput")
    embT = nc.dram_tensor("embT", [TDIM, BPC], F32, kind="ExternalInput")
    timeW = nc.dram_tensor("timeW", [TDIM, TDIM], F32, kind="ExternalInput")
    timeb = nc.dram_tensor("timeb", [TDIM, 1], F32, kind="ExternalInput")
    w0cr = nc.dram_tensor("w0cr", [4, HID], F32, kind="ExternalInput")
    wsum = nc.dram_tensor("wsum", [TDIM, HID], F32, kind="ExternalInput")
    b0d = nc.dram_tensor("b0", [128, 2], F32, kind="ExternalInput")
    # per-layer (W/3) in bf16, host-rearranged to [128, k*2 x 256] stationary layout
    wld = [nc.dram_tensor(f"w{i}", [128, 2 * HID], BF16, kind="ExternalInput") for i in (1, 2, 3)]
    bld = [nc.dram_tensor(f"b{i}", [128, 2], F32, kind="ExternalInput") for i in (1, 2, 3)]
    eyed = nc.dram_tensor("eye", [128, 128], BF16, kind="ExternalInput")
    hw1 = nc.dram_tensor("hw1", [128, 2 * HID], F32, kind="ExternalInput")
    hb1 = nc.dram_tensor("hb1", [128, 2], F32, kind="ExternalInput")
    hw2 = nc.dram_tensor("hw2", [128, 4], F32, kind="ExternalInput")
    hb2 = nc.dram_tensor("hb2", [2, 1], F32, kind="ExternalInput")
    out = nc.dram_tensor("out", [2 * BPC, V], F32, kind="ExternalOutput")

    with tile.TileContext(nc) as tc:
        with (
            tc.tile_pool(name="const", bufs=1) as pc,
            tc.tile_pool(name="hp", bufs=4) as hp,
            tc.tile_pool(name="h4p", bufs=2) as h4p,
            tc.tile_pool(name="gp", bufs=3) as gp,
            tc.tile_pool(name="hsp", bufs=3) as hsp,
            tc.tile_pool(name="t4p", bufs=4) as t4p,
            tc.tile_pool(name="h5p", bufs=2) as h5p,
            tc.tile_pool(name="op", bufs=2) as op,
            tc.tile_pool(name="ps", bufs=2, space="PSUM") as ps,
        ):
            dma = nc.sync.dma_start
            mm = nc.tensor.matmul
            act = nc.scalar.activation
            tt = nc.vector.tensor_tensor

            def ctile(shape, tag, src_ap=None, dt=F32, eng=None):
                t = pc.tile(shape, dt, tag=tag)
                if src_ap is not None:
                    d = eng.dma_start if eng is not None else dma
                    d(t[:], src_ap.bitcast(dt) if dt is F32R else src_ap)
                return t

            # critical-path constants first (layer-0 chain), bulk after.
            cCoords = ctile([128, V], "ccoords", x[:], dt=F32R)
            cEmb = ctile([TDIM, BPC], "cemb", embT[:], dt=F32R)
            cTW = ctile([TDIM, TDIM], "ctw", timeW[:], dt=F32R)
            ctb = ctile([TDIM, 1], "ctb", timeb[:])
            cWsum = ctile([TDIM, HID], "cwsum", wsum[:], dt=F32R)
            cb0 = ctile([128, 2], "cb0", b0d[:])
            cW0cR = ctile([4, HID], "cw0cr", w0cr[:], dt=F32R)
            cWl = [ctile([128, 2 * HID], f"cw{i}", wld[i][:], dt=BF16, eng=nc.gpsimd)
                   for i in range(3)]
            cBl = [ctile([128, 2], f"cbl{i}", bld[i][:], eng=nc.gpsimd) for i in range(3)]
            cI = ctile([128, 128], "ceye", eyed[:], dt=BF16, eng=nc.gpsimd)
            cHW1 = ctile([128, 2 * HID], "chw1", hw1[:], dt=F32R, eng=nc.gpsimd)
            cHB1 = ctile([128, 2], "chb1", hb1[:], eng=nc.gpsimd)
            cHW2 = ctile([128, 4], "chw2", hw2[:], dt=F32R, eng=nc.gpsimd)
            cHB2c = ctile([2, 1], "chb2c", hb2[:], eng=nc.gpsimd)

            # ---- time embedding MLP: temb = silu(emb @ time_W + time_b) ----
            pt = ps.tile([TDIM, BPC], F32, tag="ps")
            mm(pt[:], (cTW[:]), (cEmb[:]), start=True, stop=True)
            cTemb = ctile([TDIM, BPC], "ctemb", dt=F32R)
            act(cTemb[:], pt[:], AF.Silu, bias=ctb[:])

            # ---- per-sample layer-0 bias columns:
            # cb[:, m*BPC + s] = (temb_s @ (W0[2:]+res0_W[2:]) + b0)[m*128:(m+1)*128]
            cCB = ctile([128, 2 * BPC], "ccb")
            for m in range(2):
                pcb = ps.tile([128, BPC], F32, tag="ps")
                mm(pcb[:], (cWsum[:][:, m * 128:(m + 1) * 128]), (cTemb[:]),
                   start=True, stop=True)
                act(cCB[:][:, m * BPC:(m + 1) * BPC], pcb[:], AF.Identity, bias=cb0[:][:, m:m + 1])

            # ---- batched cycle-agg of coords (raw 3-term sum, no 1/3) ----
            cAggc = ctile([128, V], "caggc", dt=F32R)
            tt(cAggc[:][:, 1:1023], cCoords[:][:, 0:1022], cCoords[:][:, 2:1024], ALU.add)
            tt(cAggc[:][:, 0:1], cCoords[:][:, 1023:1024], cCoords[:][:, 1:2], ALU.add)
            tt(cAggc[:][:, 1023:1024], cCoords[:][:, 1022:1023], cCoords[:][:, 0:1], ALU.add)
            tt(cAggc[:], cAggc[:], cCoords[:], ALU.add)

            st = {}  # per-pair pipeline state; pair p = samples (2p, 2p+1)

            def halo_fix(h4d):
                dma(h4d[:, :, 0:1], h4d[:, :, 1024:1025])
                dma(h4d[:, :, 1025:1026], h4d[:, :, 1:2])

            def stage_t4(p):
                t4s = []
                for s in (2 * p, 2 * p + 1):
                    t4 = t4p.tile([4, V], F32R, tag="t4", name="t4")
                    dma(t4[0:2, :], cAggc[2 * s:2 * s + 2, :])
                    dma(t4[2:4, :], cCoords[2 * s:2 * s + 2, :])
                    t4s.append(t4)
                st[p] = {"t4": t4s}

            def stage_l0(p):
                # layer 0: h1 = silu(aggc@W0c/3 + coords@res0c + cb_s), bf16 out
                # pair h tile: [128, 4, HW], j = s*2 + m
                t4s = st[p].pop("t4")
                h = hp.tile([128, 4 * HW], BF16, tag="h", name="h")
                h4d = h[:].rearrange("p (j v) -> p j v", j=4)
                for m in range(2):
                    pw = ps.tile([128, 2 * V], F32, tag="ps", name="pc")
                    for si in range(2):
                        for c in range(2):
                            mm(pw[:][:, si * V + c * 512:si * V + (c + 1) * 512],
                               (cW0cR[:][:, m * 128:(m + 1) * 128]),
                               (t4s[si][:][:, c * 512:(c + 1) * 512]),
                               start=True, stop=True)
                    for si in range(2):
                        s = 2 * p + si
                        act(h4d[:, 2 * si + m, 1:1 + V],
                            pw[:][:, si * V:(si + 1) * V], AF.Silu,
                            bias=cCB[:][:, m * BPC + s:m * BPC + s + 1])
                halo_fix(h4d)
                st[p]["h"] = (h, h4d)

            def stage_agg(p, li):
                # aggregation prep: aligned center copy (DMA) + neighbor sum
                h, h4d = st[p]["h"]
                hs = hsp.tile([128, 4 * V], BF16, tag="hs", name="hs")
                hs4d = hs[:].rearrange("p (j v) -> p j v", j=4)
                nc.gpsimd.dma_start(hs4d, h4d[:, :, 1:1025])
                g = gp.tile([128, 4 * V], BF16, tag="g", name="g")
                g4d = g[:].rearrange("p (j v) -> p j v", j=4)
                nc.vector.tensor_tensor(g4d, h4d[:, :, 0:1024], h4d[:, :, 2:1026], ALU.add)
                st[p]["agg"] = (hs, hs4d, g, g4d)

            def stage_layer(p, li):
                # h <- silu(cycle_agg(h)@W + h + b), one K=256 matmul per layer
                h, h4d = st[p]["h"]
                cW = cWl[li]
                cB = cBl[li]
                hs, hs4d, g, g4d = st[p].pop("agg")
                nc.vector.tensor_tensor(g4d, g4d, hs4d, ALU.add)
                resid_pe = li >= 1
                last = li == 2
                if last:
                    hn = h4p.tile([128, 4 * V], F32R, tag="h4", name="h4")
                    hn4d = None
                else:
                    hn = hp.tile([128, 4 * HW], BF16, tag="h", name="h")
                    hn4d = hn[:].rearrange("p (j v) -> p j v", j=4)
                for m in range(2):
                    pw = ps.tile([128, 2 * V], F32, tag="ps", name="pc")
                    for k in range(2):
                        for si in range(2):
                            for c in range(2):
                                mm(pw[:][:, si * V + c * 512:si * V + (c + 1) * 512],
                                   (cW[:][:, k * HID + m * 128:k * HID + (m + 1) * 128]),
                                   (g[:][:, (2 * si + k) * V + c * 512:(2 * si + k) * V + (c + 1) * 512]),
                                   start=(k == 0), stop=(k == 1 and not resid_pe))
                    if resid_pe:
                        for si in range(2):
                            for c in range(2):
                                mm(pw[:][:, si * V + c * 512:si * V + (c + 1) * 512],
                                   (cI[:]),
                                   (hs[:][:, (2 * si + m) * V + c * 512:(2 * si + m) * V + (c + 1) * 512]),
                                   start=False, stop=True)
                    else:
                        nc.vector.tensor_tensor(
                            pw[:].rearrange("p (j v) -> p j v", j=2),
                            pw[:].rearrange("p (j v) -> p j v", j=2),
                            hs4d[:, m::2, :], ALU.add)
                    if last:
                        dst = hn[:].rearrange("p (j v) -> p j v", j=4)[:, m::2, :]
                    else:
                        dst = hn4d[:, m::2, 1:1 + V]
                    act(dst, pw[:], AF.Silu, bias=cB[:][:, m:m + 1])
                if not last:
                    halo_fix(hn4d)
                st[p]["h"] = (hn, hn4d)

            def stage_m1(p):
                h4, _ = st[p].pop("h")  # [128, 4, 1024], j = s*2 + k
                h5 = h5p.tile([128, 4 * V], F32R, tag="h5", name="h5")
                h54d = h5[:].rearrange("p (j v) -> p j v", j=4)
                for m in range(2):
                    pw = ps.tile([128, 2 * V], F32, tag="ps", name="pc")
                    for k in range(2):
                        for si in range(2):
                            for c in range(2):
                                mm(pw[:][:, si * V + c * 512:si * V + (c + 1) * 512],
                                   (cHW1[:][:, k * HID + m * 128:k * HID + (m + 1) * 128]),
                                   (h4[:][:, (2 * si + k) * V + c * 512:(2 * si + k) * V + (c + 1) * 512]),
                                   start=(k == 0), stop=(k == 1))
                    act(h54d[:, m::2, :], pw[:], AF.Silu, bias=cHB1[:][:, m:m + 1])
                st[p]["h5"] = h5

            def stage_m2(p):
                h5 = st[p].pop("h5")  # [128, 4, 1024], j = s*2 + m(=k)
                osb = op.tile([2, 2 * V], F32, tag="osb", name="osb")
                pm2 = ps.tile([2, 2 * V], F32, tag="ps", name="pm2")
                for k in range(2):
                    for si in range(2):
                        for c in range(2):
                            mm(pm2[:][:, si * V + c * 512:si * V + (c + 1) * 512],
                               (cHW2[:][:, 2 * k:2 * k + 2]),
                               (h5[:][:, (2 * si + k) * V + c * 512:(2 * si + k) * V + (c + 1) * 512]),
                               start=(k == 0), stop=(k == 1))
                nc.vector.tensor_scalar_add(osb[:], pm2[:], cHB2c[:])
                for si in range(2):
                    s = 2 * p + si
                    dma(out[2 * s:2 * s + 2, :], osb[:][:, si * V:(si + 1) * V])

            import os
            G = int(os.environ.get("KG", "2"))  # pairs per group
            stages = [stage_t4, stage_l0]
            for li in range(3):
                stages.append(lambda p, li=li: stage_agg(p, li))
                stages.append(lambda p, li=li: stage_layer(p, li))
            stages += [stage_m1, stage_m2]
            # software-pipeline across groups: group g runs stage st at
            # virtual time g*SKEW + st, so the next group's head stages
            # interleave with this group's MLP tail.
            NST = len(stages)
            SKEW = int(os.environ.get("KSKEW", "6"))
            NPAIR = BPC // 2
            ng = NPAIR // G
            evs = sorted((g * SKEW + sti, -sti, sti, g)
                         for g in range(ng) for sti in range(NST))
            for _, _, sti, g in evs:
                for p in range(g * G, (g + 1) * G):
                    stages[sti](p)

    nc.compile()
    return nc


def _get_prog():
    global _PROG
    if _PROG is None:
        _PROG = _build()
    return _PROG


def build_in_maps(inputs):
    f = lambda a: np.ascontiguousarray(np.asarray(a, dtype=np.float32))
    tobf = lambda a: np.ascontiguousarray(a.astype(ml_dtypes.bfloat16))
    x = f(inputs["x"])
    t = np.asarray(inputs["t"]).astype(np.int64)
    W0, b0 = f(inputs["W0"]), f(inputs["b0"])
    Ws = [f(inputs[k]) for k in ("W1", "W2", "W3")]
    bs = [f(inputs[k]) for k in ("b1", "b2", "b3")]
    res0_W = f(inputs["res0_W"])
    hW1, hb1 = f(inputs["hW1"]), f(inputs["hb1"])
    hW2, hb2 = f(inputs["hW2"]), f(inputs["hb2"])

    emb = _SIN_TABLE[t]  # (B, TDIM) gather from the constant sinusoid table

    def stat(w):  # [256, N] -> [128, 2*N] stationary layout (k-chunks in free dim)
        n = w.shape[1]
        return w.reshape(2, 128, n).transpose(1, 0, 2).reshape(128, 2 * n)

    def pbias(b):  # [256] -> [128, 2]
        return np.ascontiguousarray(b.reshape(2, 128).T)

    shared = {
        "timeW": f(inputs["time_W"]),
        "timeb": f(inputs["time_b"]).reshape(TDIM, 1),
        "w0cr": np.concatenate([W0[:2] / 3.0, res0_W[:2]], axis=0),
        "wsum": W0[2:] + res0_W[2:],
        "b0": pbias(b0),
        "eye": np.ascontiguousarray(np.eye(128, dtype=ml_dtypes.bfloat16)),
        "hw1": np.ascontiguousarray(stat(hW1)),
        "hb1": pbias(hb1),
        "hw2": np.ascontiguousarray(stat(hW2)),
        "hb2": hb2.reshape(2, 1),
    }
    for i in range(3):
        shared[f"w{i + 1}"] = tobf(stat(Ws[i] / 3.0))
        shared[f"b{i + 1}"] = pbias(bs[i])

    in_maps = []
    for c in range(NCORES):
        sl = slice(c * BPC, (c + 1) * BPC)
        m = dict(shared)
        # (BPC, 2048) -> (BPC, V, 2) -> (BPC, 2, V) -> (2*BPC, V): row 2s+c = x[s, c::2]
        m["x"] = np.ascontiguousarray(
            x[sl].reshape(BPC, V, 2).transpose(0, 2, 1).reshape(2 * BPC, V))
        m["embT"] = np.ascontiguousarray(emb[sl].T)
        in_maps.append(m)
    return in_maps


def kernel(**inputs) -> np.ndarray:
    in_maps = build_in_maps(inputs)
    nc = _get_prog()
    res = run_bass_kernel_spmd(nc, in_maps, list(range(NCORES)))
    outs = []
    for i in range(NCORES):
        o = res.results[i]["out"]  # (2*BPC, V), row 2s+c = out[s, c::2]
        outs.append(o.reshape(BPC, 2, V).transpose(0, 2, 1).reshape(BPC, DATA))
    return np.concatenate(outs, axis=0)


if __name__ == "__main__":
    rng = np.random.default_rng(0)
    demo = {
        "x": rng.standard_normal((B, DATA), dtype=np.float32),
        "t": rng.integers(0, 1000, size=(B,)).astype(np.int32),
        "time_W": rng.standard_normal((TDIM, TDIM), dtype=np.float32) / 11.3,
        "time_b": np.zeros(TDIM, np.float32),
        "W0": rng.standard_normal((130, HID), dtype=np.float32) / 11.4,
        "b0": np.zeros(HID, np.float32),
        "W1": rng.standard_normal((HID, HID), dtype=np.float32) / 16.0,
        "b1": np.zeros(HID, np.float32),
        "W2": rng.standard_normal((HID, HID), dtype=np.float32) / 16.0,
        "b2": np.zeros(HID, np.float32),
        "W3": rng.standard_normal((HID, HID), dtype=np.float32) / 16.0,
        "b3": np.zeros(HID, np.float32),
        "res0_W": rng.standard_normal((130, HID), dtype=np.float32) / 11.4,
        "hW1": rng.standard_normal((HID, HID), dtype=np.float32) / 16.0,
        "hb1": np.zeros(HID, np.float32),
        "hW2": rng.standard_normal((HID, 2), dtype=np.float32) / 16.0,
        "hb2": np.zeros(2, np.float32),
    }
    out = kernel(**demo)
    print("out", out.shape, out.dtype, float(np.abs(out).mean()))


# revision 9
# speedup vs baseline: 1.0414x; 1.0414x over previous
"""DenoiseGCN Trainium2 kernel.

Full-input contract: kernel(**inputs) takes the unsharded inputs from
setup_inputs() and returns the full (512, 2048) float32 output.

Strategy: pure data parallel over 8 NeuronCores (64 samples each, no
collectives). Activations stay resident in SBUF in a feature-major
layout ([features -> partitions, vertices -> free dim]).

v11: pair-granular SBUF tiles on top of v9 (wide-PSUM acts from the
v10 experiment starved the PE - 2 psum bufs are too few - so PSUM
stays 4x [128,1024] fp32 with per-sample acts):
  * h/g/hs tiles hold the pair ([128, 4, *] with j = s*2+m), so each
    DVE neighbor-sum TENSOR_TENSOR covers 4096 elems at 2x and the
    halo-fix / center-copy DMA count halves.
  * matmul loops are k-outer so consecutive matmuls share the
    stationary operand (fewer LDWEIGHTS in the PE stream).
"""

import numpy as np
import ml_dtypes

import concourse.bacc as bacc
import concourse.mybir as mybir
import concourse.tile as tile
from concourse.bass_utils import run_bass_kernel_spmd

F32 = mybir.dt.float32
F32R = mybir.dt.float32r
BF16 = mybir.dt.bfloat16
AF = mybir.ActivationFunctionType
ALU = mybir.AluOpType

NCORES = 8
B = 512
BPC = B // NCORES  # samples per core
V = 1024           # vertices per sample
HID = 256
TDIM = 128
DATA = 2048
HW = 1026          # haloed row width per feature-tile (1 + 1024 + 1)

def _sin_table():
    half = TDIM // 2
    freqs = np.exp(-np.log(10000.0) * np.arange(half, dtype=np.float64) / (half - 1))
    tt = np.arange(1000, dtype=np.float64)[:, None] * freqs[None, :]
    return np.concatenate([np.sin(tt), np.cos(tt)], axis=1).astype(np.float32)


_SIN_TABLE = _sin_table()

_PROG = None


def _build():
    nc = bacc.Bacc("TRN2", target_bir_lowering=False, debug=False, num_devices=NCORES)

    x = nc.dram_tensor("x", [2 * BPC, V], F32, kind="ExternalInput")
    embT = nc.dram_tensor("embT", [TDIM, BPC], F32, kind="ExternalInput")
    timeW = nc.dram_tensor("timeW", [TDIM, TDIM], F32, kind="ExternalInput")
    timeb = nc.dram_tensor("timeb", [TDIM, 1], F32, kind="ExternalInput")
    w0cr = nc.dram_tensor("w0cr", [4, HID], F32, kind="ExternalInput")
    wsum = nc.dram_tensor("wsum", [TDIM, HID], F32, kind="ExternalInput")
    b0d = nc.dram_tensor("b0", [128, 2], F32, kind="ExternalInput")
    # per-layer (W/3) in bf16, host-rearranged to [128, k*2 x 256] stationary layout
    wld = [nc.dram_tensor(f"w{i}", [128, 2 * HID], BF16, kind="ExternalInput") for i in (1, 2, 3)]
    bld = [nc.dram_tensor(f"b{i}", [128, 2], F32, kind="ExternalInput") for i in (1, 2, 3)]
    eyed = nc.dram_tensor("eye", [128, 128], BF16, kind="ExternalInput")
    hw1 = nc.dram_tensor("hw1", [128, 2 * HID], F32, kind="ExternalInput")
    hb1 = nc.dram_tensor("hb1", [128, 2], F32, kind="ExternalInput")
    hw2 = nc.dram_tensor("hw2", [128, 4], F32, kind="ExternalInput")
    hb2 = nc.dram_tensor("hb2", [2, 1], F32, kind="ExternalInput")
    out = nc.dram_tensor("out", [2 * BPC, V], F32, kind="ExternalOutput")

    with tile.TileContext(nc) as tc:
        with (
            tc.tile_pool(name="const", bufs=1) as pc,
            tc.tile_pool(name="hp", bufs=4) as hp,
            tc.tile_pool(name="h4p", bufs=2) as h4p,
            tc.tile_pool(name="gp", bufs=3) as gp,
            tc.tile_pool(name="hsp", bufs=3) as hsp,
            tc.tile_pool(name="t4p", bufs=4) as t4p,
            tc.tile_pool(name="h5p", bufs=2) as h5p,
            tc.tile_pool(name="op", bufs=2) as op,
            tc.tile_pool(name="ps", bufs=4, space="PSUM") as ps,
        ):
            dma = nc.sync.dma_start
            mm = nc.tensor.matmul
            act = nc.scalar.activation
            tt = nc.vector.tensor_tensor

            def ctile(shape, tag, src_ap=None, dt=F32, eng=None):
                t = pc.tile(shape, dt, tag=tag)
                if src_ap is not None:
                    d = eng.dma_start if eng is not None else dma
                    d(t[:], src_ap.bitcast(dt) if dt is F32R else src_ap)
                return t

            # critical-path constants first (layer-0 chain), bulk after.
            cCoords = ctile([128, V], "ccoords", x[:], dt=F32R)
            cEmb = ctile([TDIM, BPC], "cemb", embT[:], dt=F32R)
            cTW = ctile([TDIM, TDIM], "ctw", timeW[:], dt=F32R)
            ctb = ctile([TDIM, 1], "ctb", timeb[:])
            cWsum = ctile([TDIM, HID], "cwsum", wsum[:], dt=F32R)
            cb0 = ctile([128, 2], "cb0", b0d[:])
            cW0cR = ctile([4, HID], "cw0cr", w0cr[:], dt=F32R)
            cWl = [ctile([128, 2 * HID], f"cw{i}", wld[i][:], dt=BF16, eng=nc.gpsimd)
                   for i in range(3)]
            cBl = [ctile([128, 2], f"cbl{i}", bld[i][:], eng=nc.gpsimd) for i in range(3)]
            cI = ctile([128, 128], "ceye", eyed[:], dt=BF16, eng=nc.gpsimd)
            cHW1 = ctile([128, 2 * HID], "chw1", hw1[:], dt=F32R, eng=nc.gpsimd)
            cHB1 = ctile([128, 2], "chb1", hb1[:], eng=nc.gpsimd)
            cHW2 = ctile([128, 4], "chw2", hw2[:], dt=F32R, eng=nc.gpsimd)
            cHB2c = ctile([2, 1], "chb2c", hb2[:], eng=nc.gpsimd)

            # ---- time embedding MLP: temb = silu(emb @ time_W + time_b) ----
            pt = ps.tile([TDIM, BPC], F32, tag="ps")
            mm(pt[:], (cTW[:]), (cEmb[:]), start=True, stop=True)
            cTemb = ctile([TDIM, BPC], "ctemb", dt=F32R)
            act(cTemb[:], pt[:], AF.Silu, bias=ctb[:])

            # ---- per-sample layer-0 bias columns:
            # cb[:, m*BPC + s] = (temb_s @ (W0[2:]+res0_W[2:]) + b0)[m*128:(m+1)*128]
            cCB = ctile([128, 2 * BPC], "ccb")
            for m in range(2):
                pcb = ps.tile([128, BPC], F32, tag="ps")
                mm(pcb[:], (cWsum[:][:, m * 128:(m + 1) * 128]), (cTemb[:]),
                   start=True, stop=True)
                act(cCB[:][:, m * BPC:(m + 1) * BPC], pcb[:], AF.Identity, bias=cb0[:][:, m:m + 1])

            # ---- batched cycle-agg of coords (raw 3-term sum, no 1/3) ----
            cAggc = ctile([128, V], "caggc", dt=F32R)
            tt(cAggc[:][:, 1:1023], cCoords[:][:, 0:1022], cCoords[:][:, 2:1024], ALU.add)
            tt(cAggc[:][:, 0:1], cCoords[:][:, 1023:1024], cCoords[:][:, 1:2], ALU.add)
            tt(cAggc[:][:, 1023:1024], cCoords[:][:, 1022:1023], cCoords[:][:, 0:1], ALU.add)
            tt(cAggc[:], cAggc[:], cCoords[:], ALU.add)

            st = {}  # per-pair pipeline state; pair p = samples (2p, 2p+1)

            def halo_fix(h4d):
                dma(h4d[:, :, 0:1], h4d[:, :, 1024:1025])
                dma(h4d[:, :, 1025:1026], h4d[:, :, 1:2])

            def stage_t4(p):
                t4s = []
                for s in (2 * p, 2 * p + 1):
                    t4 = t4p.tile([4, V], F32R, tag="t4", name="t4")
                    dma(t4[0:2, :], cAggc[2 * s:2 * s + 2, :])
                    dma(t4[2:4, :], cCoords[2 * s:2 * s + 2, :])
                    t4s.append(t4)
                st[p] = {"t4": t4s}

            def stage_l0(p):
                # layer 0: h1 = silu(aggc@W0c/3 + coords@res0c + cb_s), bf16 out
                # pair h tile: [128, 4, HW], j = s*2 + m
                t4s = st[p].pop("t4")
                h = hp.tile([128, 4 * HW], BF16, tag="h", name="h")
                h4d = h[:].rearrange("p (j v) -> p j v", j=4)
                for m in range(2):
                    pws = [ps.tile([128, V], F32, tag="ps", name="pc")
                           for _ in range(2)]
                    for si in range(2):
                        for c in range(2):
                            mm(pws[si][:][:, c * 512:(c + 1) * 512],
                               (cW0cR[:][:, m * 128:(m + 1) * 128]),
                               (t4s[si][:][:, c * 512:(c + 1) * 512]),
                               start=True, stop=True)
                    for si in range(2):
                        s = 2 * p + si
                        act(h4d[:, 2 * si + m, 1:1 + V],
                            pws[si][:], AF.Silu,
                            bias=cCB[:][:, m * BPC + s:m * BPC + s + 1])
                halo_fix(h4d)
                st[p]["h"] = (h, h4d)

            def stage_agg(p, li):
                # aggregation prep: aligned center copy (DMA) + neighbor sum
                h, h4d = st[p]["h"]
                hs = hsp.tile([128, 4 * V], BF16, tag="hs", name="hs")
                hs4d = hs[:].rearrange("p (j v) -> p j v", j=4)
                nc.gpsimd.dma_start(hs4d, h4d[:, :, 1:1025])
                g = gp.tile([128, 4 * V], BF16, tag="g", name="g")
                g4d = g[:].rearrange("p (j v) -> p j v", j=4)
                nc.vector.tensor_tensor(g4d, h4d[:, :, 0:1024], h4d[:, :, 2:1026], ALU.add)
                st[p]["agg"] = (hs, hs4d, g, g4d)

            def stage_layer(p, li):
                # h <- silu(cycle_agg(h)@W + h + b), one K=256 matmul per layer
                h, h4d = st[p]["h"]
                cW = cWl[li]
                cB = cBl[li]
                hs, hs4d, g, g4d = st[p].pop("agg")
                nc.vector.tensor_tensor(g4d, g4d, hs4d, ALU.add)
                resid_pe = li >= 1
                last = li == 2
                if last:
                    hn = h4p.tile([128, 4 * V], F32R, tag="h4", name="h4")
                    hn4d = None
                else:
                    hn = hp.tile([128, 4 * HW], BF16, tag="h", name="h")
                    hn4d = hn[:].rearrange("p (j v) -> p j v", j=4)
                for m in range(2):
                    pws = [ps.tile([128, V], F32, tag="ps", name="pc")
                           for _ in range(2)]
                    for k in range(2):
                        for si in range(2):
                            for c in range(2):
                                mm(pws[si][:][:, c * 512:(c + 1) * 512],
                                   (cW[:][:, k * HID + m * 128:k * HID + (m + 1) * 128]),
                                   (g[:][:, (2 * si + k) * V + c * 512:(2 * si + k) * V + (c + 1) * 512]),
                                   start=(k == 0), stop=(k == 1 and not resid_pe))
                    if resid_pe:
                        for si in range(2):
                            for c in range(2):
                                mm(pws[si][:][:, c * 512:(c + 1) * 512],
                                   (cI[:]),
                                   (hs[:][:, (2 * si + m) * V + c * 512:(2 * si + m) * V + (c + 1) * 512]),
                                   start=False, stop=True)
                    for si in range(2):
                        if not resid_pe:
                            nc.vector.tensor_tensor(
                                pws[si][:], pws[si][:],
                                hs4d[:, 2 * si + m, :], ALU.add)
                        if last:
                            dst = hn[:].rearrange("p (j v) -> p j v", j=4)[:, 2 * si + m, :]
                        else:
                            dst = hn4d[:, 2 * si + m, 1:1 + V]
                        act(dst, pws[si][:], AF.Silu, bias=cB[:][:, m:m + 1])
                if not last:
                    halo_fix(hn4d)
                st[p]["h"] = (hn, hn4d)

            def stage_m1(p):
                h4, _ = st[p].pop("h")  # [128, 4, 1024], j = s*2 + k
                h5 = h5p.tile([128, 4 * V], F32R, tag="h5", name="h5")
                h54d = h5[:].rearrange("p (j v) -> p j v", j=4)
                for m in range(2):
                    pws = [ps.tile([128, V], F32, tag="ps", name="pc")
                           for _ in range(2)]
                    for k in range(2):
                        for si in range(2):
                            for c in range(2):
                                mm(pws[si][:][:, c * 512:(c + 1) * 512],
                                   (cHW1[:][:, k * HID + m * 128:k * HID + (m + 1) * 128]),
                                   (h4[:][:, (2 * si + k) * V + c * 512:(2 * si + k) * V + (c + 1) * 512]),
                                   start=(k == 0), stop=(k == 1))
                    for si in range(2):
                        act(h54d[:, 2 * si + m, :], pws[si][:], AF.Silu,
                            bias=cHB1[:][:, m:m + 1])
                st[p]["h5"] = h5

            def stage_m2(p):
                h5 = st[p].pop("h5")  # [128, 4, 1024], j = s*2 + m(=k)
                osb = op.tile([2, 2 * V], F32, tag="osb", name="osb")
                pms = [ps.tile([2, V], F32, tag="ps", name="pm2")
                       for _ in range(2)]
                for k in range(2):
                    for si in range(2):
                        for c in range(2):
                            mm(pms[si][:][:, c * 512:(c + 1) * 512],
                               (cHW2[:][:, 2 * k:2 * k + 2]),
                               (h5[:][:, (2 * si + k) * V + c * 512:(2 * si + k) * V + (c + 1) * 512]),
                               start=(k == 0), stop=(k == 1))
                for si in range(2):
                    nc.vector.tensor_scalar_add(osb[:][:, si * V:(si + 1) * V],
                                                pms[si][:], cHB2c[:])
                for si in range(2):
                    s = 2 * p + si
                    dma(out[2 * s:2 * s + 2, :], osb[:][:, si * V:(si + 1) * V])

            import os
            G = int(os.environ.get("KG", "2"))  # pairs per group
            stages = [stage_t4, stage_l0]
            for li in range(3):
                stages.append(lambda p, li=li: stage_agg(p, li))
                stages.append(lambda p, li=li: stage_layer(p, li))
            stages += [stage_m1, stage_m2]
            # software-pipeline across groups: group g runs stage st at
            # virtual time g*SKEW + st, so the next group's head stages
            # interleave with this group's MLP tail.
            NST = len(stages)
            SKEW = int(os.environ.get("KSKEW", "6"))
            NPAIR = BPC // 2
            ng = NPAIR // G
            evs = sorted((g * SKEW + sti, -sti, sti, g)
                         for g in range(ng) for sti in range(NST))
            for _, _, sti, g in evs:
                for p in range(g * G, (g + 1) * G):
                    stages[sti](p)

    nc.compile()
    return nc


def _get_prog():
    global _PROG
    if _PROG is None:
        _PROG = _build()
    return _PROG


def build_in_maps(inputs):
    f = lambda a: np.ascontiguousarray(np.asarray(a, dtype=np.float32))
    tobf = lambda a: np.ascontiguousarray(a.astype(ml_dtypes.bfloat16))
    x = f(inputs["x"])
    t = np.asarray(inputs["t"]).astype(np.int64)
    W0, b0 = f(inputs["W0"]), f(inputs["b0"])
    Ws = [f(inputs[k]) for k in ("W1", "W2", "W3")]
    bs = [f(inputs[k]) for k in ("b1", "b2", "b3")]
    res0_W = f(inputs["res0_W"])
    hW1, hb1 = f(inputs["hW1"]), f(inputs["hb1"])
    hW2, hb2 = f(inputs["hW2"]), f(inputs["hb2"])

    emb = _SIN_TABLE[t]  # (B, TDIM) gather from the constant sinusoid table

    def stat(w):  # [256, N] -> [128, 2*N] stationary layout (k-chunks in free dim)
        n = w.shape[1]
        return w.reshape(2, 128, n).transpose(1, 0, 2).reshape(128, 2 * n)

    def pbias(b):  # [256] -> [128, 2]
        return np.ascontiguousarray(b.reshape(2, 128).T)

    shared = {
        "timeW": f(inputs["time_W"]),
        "timeb": f(inputs["time_b"]).reshape(TDIM, 1),
        "w0cr": np.concatenate([W0[:2] / 3.0, res0_W[:2]], axis=0),
        "wsum": W0[2:] + res0_W[2:],
        "b0": pbias(b0),
        "eye": np.ascontiguousarray(np.eye(128, dtype=ml_dtypes.bfloat16)),
        "hw1": np.ascontiguousarray(stat(hW1)),
        "hb1": pbias(hb1),
        "hw2": np.ascontiguousarray(stat(hW2)),
        "hb2": hb2.reshape(2, 1),
    }
    for i in range(3):
        shared[f"w{i + 1}"] = tobf(stat(Ws[i] / 3.0))
        shared[f"b{i + 1}"] = pbias(bs[i])

    in_maps = []
    for c in range(NCORES):
        sl = slice(c * BPC, (c + 1) * BPC)
        m = dict(shared)
        # (BPC, 2048) -> (BPC, V, 2) -> (BPC, 2, V) -> (2*BPC, V): row 2s+c = x[s, c::2]
        m["x"] = np.ascontiguousarray(
            x[sl].reshape(BPC, V, 2).transpose(0, 2, 1).reshape(2 * BPC, V))
        m["embT"] = np.ascontiguousarray(emb[sl].T)
        in_maps.append(m)
    return in_maps


def kernel(**inputs) -> np.ndarray:
    in_maps = build_in_maps(inputs)
    nc = _get_prog()
    res = run_bass_kernel_spmd(nc, in_maps, list(range(NCORES)))
    outs = []
    for i in range(NCORES):
        o = res.results[i]["out"]  # (2*BPC, V), row 2s+c = out[s, c::2]
        outs.append(o.reshape(BPC, 2, V).transpose(0, 2, 1).reshape(BPC, DATA))
    return np.concatenate(outs, axis=0)


if __name__ == "__main__":
    rng = np.random.default_rng(0)
    demo = {
        "x": rng.standard_normal((B, DATA), dtype=np.float32),
        "t": rng.integers(0, 1000, size=(B,)).astype(np.int32),
        "time_W": rng.standard_normal((TDIM, TDIM), dtype=np.float32) / 11.3,
        "time_b": np.zeros(TDIM, np.float32),
        "W0": rng.standard_normal((130, HID), dtype=np.float32) / 11.4,
        "b0": np.zeros(HID, np.float32),
        "W1": rng.standard_normal((HID, HID), dtype=np.float32) / 16.0,
        "b1": np.zeros(HID, np.float32),
        "W2": rng.standard_normal((HID, HID), dtype=np.float32) / 16.0,
        "b2": np.zeros(HID, np.float32),
        "W3": rng.standard_normal((HID, HID), dtype=np.float32) / 16.0,
        "b3": np.zeros(HID, np.float32),
        "res0_W": rng.standard_normal((130, HID), dtype=np.float32) / 11.4,
        "hW1": rng.standard_normal((HID, HID), dtype=np.float32) / 16.0,
        "hb1": np.zeros(HID, np.float32),
        "hW2": rng.standard_normal((HID, 2), dtype=np.float32) / 16.0,
        "hb2": np.zeros(2, np.float32),
    }
    out = kernel(**demo)
    print("out", out.shape, out.dtype, float(np.abs(out).mean()))


# revision 24
# speedup vs baseline: 1.1038x; 1.0599x over previous
"""DenoiseGCN Trainium2 kernel.

Full-input contract: kernel(**inputs) takes the unsharded inputs from
setup_inputs() and returns the full (512, 2048) float32 output.

Strategy: pure data parallel over 8 NeuronCores (64 samples each, no
collectives). Activations stay resident in SBUF in a feature-major
layout ([features -> partitions, vertices -> free dim]).

Final (v9, 1011.6us HW exec, abs-max rel err 8.8e-3) vs the 1.07ms
baseline (which ran TWO K=256 matmuls per GCN layer, tensor-bound at 97%):
  * stage emission is software-pipelined ACROSS sample groups: group g
    runs stage st at virtual time g*SKEW + st (SKEW=6 over 10 stages),
    so the next group's input/layer-0 head overlaps this group's MLP
    tail - group-sequential emission cost ~7us of PE drain per group.
  * one PSUM pool of four [128,1024] buffers (all 8 banks; the head
    stage's tiny [2,512] tiles allocate from the same pool).
  * each layer now runs ONE K=256 matmul: p = g' @ (W/3) with
    g' = h[v-1] + h[v] + h[v+1] built by two bf16 tensor_tensor adds on
    the vector engine (bf16 SBUF operands hit the DVE 2x_1p mode).
  * the residual +h is added into PSUM either by a bf16 identity-matrix
    matmul on the tensor engine (start=False accumulate) or by a DVE
    tensor_tensor on PSUM, chosen per (sample, layer) to balance engines.
  * body activations/weights are bf16 (abs-max rel err ~1e-2 vs 2e-2
    budget); layer-0, the residual accumulation (fp32 PSUM), and the
    whole MLP head stay fp32r to keep the error down.
  * silu(psum + bias) fused on the scalar engine per m-chunk
    ([128,1024] PSUM reads - fewer, larger activations than baseline).
"""

import numpy as np
import ml_dtypes

import concourse.bacc as bacc
import concourse.mybir as mybir
import concourse.tile as tile
from concourse.bass_utils import run_bass_kernel_spmd

F32 = mybir.dt.float32
F32R = mybir.dt.float32r
BF16 = mybir.dt.bfloat16
AF = mybir.ActivationFunctionType
ALU = mybir.AluOpType

NCORES = 8
B = 512
BPC = B // NCORES  # samples per core
V = 1024           # vertices per sample
HID = 256
TDIM = 128
DATA = 2048
HW = 1026          # haloed row width per feature-tile (1 + 1024 + 1)

def _sin_table():
    half = TDIM // 2
    freqs = np.exp(-np.log(10000.0) * np.arange(half, dtype=np.float64) / (half - 1))
    tt = np.arange(1000, dtype=np.float64)[:, None] * freqs[None, :]
    return np.concatenate([np.sin(tt), np.cos(tt)], axis=1).astype(np.float32)


_SIN_TABLE = _sin_table()

_PROG = None


def _build():
    nc = bacc.Bacc("TRN2", target_bir_lowering=False, debug=False, num_devices=NCORES)

    x = nc.dram_tensor("x", [2 * BPC, V], F32, kind="ExternalInput")
    embT = nc.dram_tensor("embT", [TDIM, BPC], F32, kind="ExternalInput")
    timeW = nc.dram_tensor("timeW", [TDIM, TDIM], F32, kind="ExternalInput")
    timeb = nc.dram_tensor("timeb", [TDIM, 1], F32, kind="ExternalInput")
    w0cr = nc.dram_tensor("w0cr", [128, HID], F32, kind="ExternalInput")
    wsum = nc.dram_tensor("wsum", [TDIM, HID], F32, kind="ExternalInput")
    b0d = nc.dram_tensor("b0", [128, 2], F32, kind="ExternalInput")
    # per-layer (W/3) in bf16, host-rearranged to [128, k*2 x 256] stationary layout
    wld = [nc.dram_tensor(f"w{i}", [128, 2 * HID], BF16, kind="ExternalInput") for i in (1, 2, 3)]
    bld = [nc.dram_tensor(f"b{i}", [128, 2], F32, kind="ExternalInput") for i in (1, 2, 3)]
    eyed = nc.dram_tensor("eye", [128, 128], BF16, kind="ExternalInput")
    hw1 = nc.dram_tensor("hw1", [128, 2 * HID], F32, kind="ExternalInput")
    hb1 = nc.dram_tensor("hb1", [128, 2], F32, kind="ExternalInput")
    hw2 = nc.dram_tensor("hw2", [128, 64], F32, kind="ExternalInput")
    hb2 = nc.dram_tensor("hb2", [2, 1], F32, kind="ExternalInput")
    out = nc.dram_tensor("out", [2 * BPC, V], F32, kind="ExternalOutput")

    with tile.TileContext(nc) as tc:
        with (
            tc.tile_pool(name="const", bufs=1) as pc,
            tc.tile_pool(name="hp", bufs=8) as hp,
            tc.tile_pool(name="h4p", bufs=4) as h4p,
            tc.tile_pool(name="gp", bufs=6) as gp,
            tc.tile_pool(name="hsp", bufs=6) as hsp,
            tc.tile_pool(name="t4p", bufs=6) as t4p,
            tc.tile_pool(name="h5p", bufs=4) as h5p,
            tc.tile_pool(name="op", bufs=4) as op,
            tc.tile_pool(name="ps", bufs=4, space="PSUM") as ps,
        ):
            dma = nc.sync.dma_start
            mm = nc.tensor.matmul
            act = nc.scalar.activation
            tt = nc.vector.tensor_tensor

            def ctile(shape, tag, src_ap=None, dt=F32, eng=None):
                t = pc.tile(shape, dt, tag=tag)
                if src_ap is not None:
                    d = eng.dma_start if eng is not None else dma
                    d(t[:], src_ap.bitcast(dt) if dt is F32R else src_ap)
                return t

            # critical-path constants first (layer-0 chain), bulk after.
            cCoords = ctile([128, V], "ccoords", x[:], dt=F32R)
            cEmb = ctile([TDIM, BPC], "cemb", embT[:], dt=F32R)
            cTW = ctile([TDIM, TDIM], "ctw", timeW[:], dt=F32R)
            ctb = ctile([TDIM, 1], "ctb", timeb[:])
            cWsum = ctile([TDIM, HID], "cwsum", wsum[:], dt=F32R)
            cb0 = ctile([128, 2], "cb0", b0d[:])
            cW0cR = ctile([128, HID], "cw0cr", w0cr[:], dt=F32R)
            cWl = [ctile([128, 2 * HID], f"cw{i}", wld[i][:], dt=BF16, eng=nc.gpsimd)
                   for i in range(3)]
            cBl = [ctile([128, 2], f"cbl{i}", bld[i][:], eng=nc.gpsimd) for i in range(3)]
            cI = ctile([128, 128], "ceye", eyed[:], dt=BF16, eng=nc.gpsimd)
            cHW1 = ctile([128, 2 * HID], "chw1", hw1[:], dt=F32R, eng=nc.gpsimd)
            cHB1 = ctile([128, 2], "chb1", hb1[:], eng=nc.gpsimd)
            cHW2 = ctile([128, 64], "chw2", hw2[:], dt=F32R, eng=nc.gpsimd)
            cHB2c = ctile([2, 1], "chb2c", hb2[:], eng=nc.gpsimd)

            # ---- time embedding MLP: temb = silu(emb @ time_W + time_b) ----
            pt = ps.tile([TDIM, BPC], F32, tag="ps")
            mm(pt[:], (cTW[:]), (cEmb[:]), start=True, stop=True)
            cTemb = ctile([TDIM, BPC], "ctemb", dt=F32R)
            act(cTemb[:], pt[:], AF.Silu, bias=ctb[:])

            # ---- per-sample layer-0 bias columns:
            # cb[:, m*BPC + s] = (temb_s @ (W0[2:]+res0_W[2:]) + b0)[m*128:(m+1)*128]
            cCB = ctile([128, 2 * BPC], "ccb")
            for m in range(2):
                pcb = ps.tile([128, BPC], F32, tag="ps")
                mm(pcb[:], (cWsum[:][:, m * 128:(m + 1) * 128]), (cTemb[:]),
                   start=True, stop=True)
                act(cCB[:][:, m * BPC:(m + 1) * BPC], pcb[:], AF.Identity, bias=cb0[:][:, m:m + 1])

            # ---- batched cycle-agg of coords (raw 3-term sum, no 1/3) ----
            cAggc = ctile([128, V], "caggc", dt=F32R)
            tt(cAggc[:][:, 1:1023], cCoords[:][:, 0:1022], cCoords[:][:, 2:1024], ALU.add)
            tt(cAggc[:][:, 0:1], cCoords[:][:, 1023:1024], cCoords[:][:, 1:2], ALU.add)
            tt(cAggc[:][:, 1023:1024], cCoords[:][:, 1022:1023], cCoords[:][:, 0:1], ALU.add)
            tt(cAggc[:], cAggc[:], cCoords[:], ALU.add)

            st = {}  # per-sample pipeline state

            def halo_fix(s, li, h, h3):
                dma(h3[:, :, 0:1], h3[:, :, 1024:1025])
                dma(h3[:, :, 1025:1026], h3[:, :, 1:2])

            def stage_t4(s):
                # t4 rows {32j..32j+3} = [aggc(2), coords(2)] replicated at
                # the four 32-row bases so layer-0's four K=4 matmuls can
                # row-pack onto distinct PE row groups and run concurrently.
                t4 = t4p.tile([128, V], F32R, tag="t4", name="t4")
                for j in range(4):
                    d = dma if j < 2 else nc.gpsimd.dma_start
                    d(t4[32 * j:32 * j + 2, :], cAggc[2 * s:2 * s + 2, :])
                    d(t4[32 * j + 2:32 * j + 4, :], cCoords[2 * s:2 * s + 2, :])
                st[s] = {"t4": t4}

            def stage_l0(s):
                # layer 0: h1 = silu(aggc@W0c/3 + coords@res0c + cb_s), bf16 out
                # four K=4 matmuls row-packed at bases {0,32,64,96}.
                t4 = st[s].pop("t4")
                h = hp.tile([128, 2 * HW], BF16, tag="h", name="h")
                h3 = h[:].rearrange("p (m v) -> p m v", m=2)
                pws = [ps.tile([128, 2 * 512], F32, tag="ps", name="pc")
                       for _ in range(2)]
                for m in range(2):
                    for c in range(2):
                        j = 2 * m + c
                        mm(pws[m][:][:, c * 512:(c + 1) * 512],
                           (cW0cR[32 * j:32 * j + 4, m * 128:(m + 1) * 128]),
                           (t4[32 * j:32 * j + 4, c * 512:(c + 1) * 512]),
                           start=True, stop=True, tile_position=(32 * j, 0))
                for m in range(2):
                    act(h[:][:, m * HW + 1:m * HW + 1 + V],
                        pws[m][:], AF.Silu,
                        bias=cCB[:][:, m * BPC + s:m * BPC + s + 1])
                halo_fix(s, 0, h, h3)
                st[s]["h"] = (h, h3)

            def stage_agg(s, li):
                # aggregation prep: aligned center copy (DMA) + neighbor sum
                h, h3 = st[s]["h"]
                hs = hsp.tile([128, 2 * V], BF16, tag="hs", name="hs")
                hs3 = hs[:].rearrange("p (m v) -> p m v", m=2)
                nc.gpsimd.dma_start(hs3, h3[:, :, 1:1025])
                g = gp.tile([128, 2 * V], BF16, tag="g", name="g")
                g3 = g[:].rearrange("p (m v) -> p m v", m=2)
                nc.vector.tensor_tensor(g3, h3[:, :, 0:1024], h3[:, :, 2:1026], ALU.add)
                st[s]["agg"] = (hs, hs3, g, g3)

            def stage_layer(s, li):
                # h <- silu(cycle_agg(h)@W + h + b), one K=256 matmul per layer
                h, h3 = st[s]["h"]
                cW = cWl[li]
                cB = cBl[li]
                hs, hs3, g, g3 = st[s].pop("agg")
                nc.vector.tensor_tensor(g3, g3, hs3, ALU.add)
                resid_pe = li >= 1
                last = li == 2
                if last:
                    hn = h4p.tile([128, 2 * V], F32R, tag="h4", name="h4")
                    hn3 = None
                else:
                    hn = hp.tile([128, 2 * HW], BF16, tag="h", name="h")
                    hn3 = hn[:].rearrange("p (m v) -> p m v", m=2)
                for m in range(2):
                    p = ps.tile([128, 2 * 512], F32, tag="ps", name="pc")
                    for c in range(2):
                        pcol = p[:][:, c * 512:(c + 1) * 512]
                        for k in range(2):
                            mm(pcol, (cW[:][:, k * HID + m * 128:k * HID + (m + 1) * 128]),
                               (g[:][:, k * V + c * 512:k * V + (c + 1) * 512]),
                               start=(k == 0), stop=(k == 1 and not resid_pe))
                        if resid_pe:
                            mm(pcol, (cI[:]),
                               (hs[:][:, m * V + c * 512:m * V + (c + 1) * 512]),
                               start=False, stop=True)
                    if not resid_pe:
                        nc.vector.tensor_tensor(
                            p[:], p[:], hs3[:, m:m + 1, :], ALU.add)
                    dst = (hn[:][:, m * V:(m + 1) * V] if last
                           else hn[:][:, m * HW + 1:m * HW + 1 + V])
                    act(dst, p[:], AF.Silu, bias=cB[:][:, m:m + 1])
                if not last:
                    halo_fix(s, li + 1, hn, hn3)
                st[s]["h"] = (hn, hn3)

            def stage_m1(s):
                h4, _ = st[s].pop("h")
                h5 = h5p.tile([128, 2 * V], F32R, tag="h5", name="h5")
                for m in range(2):
                    p = ps.tile([128, 2 * 512], F32, tag="ps", name="pc")
                    for c in range(2):
                        pcol = p[:][:, c * 512:(c + 1) * 512]
                        for k in range(2):
                            mm(pcol, (cHW1[:][:, k * HID + m * 128:k * HID + (m + 1) * 128]),
                               (h4[:][:, k * V + c * 512:k * V + (c + 1) * 512]),
                               start=(k == 0), stop=(k == 1))
                    act(h5[:][:, m * V:(m + 1) * V], p[:], AF.Silu, bias=cHB1[:][:, m:m + 1])
                st[s]["h5"] = h5

            def stage_m2(s):
                h5 = st[s].pop("h5")
                osb = op.tile([2, V], F32, tag="osb", name="osb")
                for c in range(2):
                    pm2 = ps.tile([2, 512], F32, tag="ps", name="pm2")
                    mm(pm2[:], (cHW2[:][:, 0:2]), (h5[:][:, c * 512:(c + 1) * 512]),
                       start=True, stop=False)
                    mm(pm2[:], (cHW2[:][:, 32:34]), (h5[:][:, V + c * 512:V + (c + 1) * 512]),
                       start=False, stop=True)
                    nc.vector.tensor_scalar_add(osb[:][:, c * 512:(c + 1) * 512],
                                                pm2[:], cHB2c[:])
                dma(out[2 * s:2 * s + 2, :], osb[:])

            import os
            G = int(os.environ.get("KG", "4"))
            stages = [stage_t4, stage_l0]
            for li in range(3):
                stages.append(lambda s, li=li: stage_agg(s, li))
                stages.append(lambda s, li=li: stage_layer(s, li))
            stages += [stage_m1, stage_m2]
            # software-pipeline across groups: group g runs stage st at
            # time g*SKEW + st, so the next group's t4/l0 stages interleave
            # with this group's m1/m2 tail instead of waiting for it (the
            # group-sequential schedule showed a ~7us PE drain per group).
            NST = len(stages)
            SKEW = int(os.environ.get("KSKEW", "6"))
            ng = BPC // G
            evs = sorted((g * SKEW + sti, -sti, sti, g)
                         for g in range(ng) for sti in range(NST))
            for _, _, sti, g in evs:
                for s in range(g * G, (g + 1) * G):
                    stages[sti](s)

    nc.compile()
    return nc


def _get_prog():
    global _PROG
    if _PROG is None:
        _PROG = _build()
    return _PROG


def _hw2pad(s):  # [128, 2k x 2] -> [128, 2k x 32] (M zero-padded to 32)
    w = np.zeros((128, 64), np.float32)
    for k in range(2):
        w[:, 32 * k:32 * k + 2] = s[:, 2 * k:2 * k + 2]
    return w


def _w0c4(base):  # [4, HID] -> [128, HID] with base at rows {32j..32j+3}
    w = np.zeros((128, HID), np.float32)
    for j in range(4):
        w[32 * j:32 * j + 4] = base
    return w


def build_in_maps(inputs):
    f = lambda a: np.ascontiguousarray(np.asarray(a, dtype=np.float32))
    tobf = lambda a: np.ascontiguousarray(a.astype(ml_dtypes.bfloat16))
    x = f(inputs["x"])
    t = np.asarray(inputs["t"]).astype(np.int64)
    W0, b0 = f(inputs["W0"]), f(inputs["b0"])
    Ws = [f(inputs[k]) for k in ("W1", "W2", "W3")]
    bs = [f(inputs[k]) for k in ("b1", "b2", "b3")]
    res0_W = f(inputs["res0_W"])
    hW1, hb1 = f(inputs["hW1"]), f(inputs["hb1"])
    hW2, hb2 = f(inputs["hW2"]), f(inputs["hb2"])

    emb = _SIN_TABLE[t]  # (B, TDIM) gather from the constant sinusoid table

    def stat(w):  # [256, N] -> [128, 2*N] stationary layout (k-chunks in free dim)
        n = w.shape[1]
        return w.reshape(2, 128, n).transpose(1, 0, 2).reshape(128, 2 * n)

    def pbias(b):  # [256] -> [128, 2]
        return np.ascontiguousarray(b.reshape(2, 128).T)

    shared = {
        "timeW": f(inputs["time_W"]),
        "timeb": f(inputs["time_b"]).reshape(TDIM, 1),
        "w0cr": _w0c4(np.concatenate([W0[:2] / 3.0, res0_W[:2]], axis=0)),
        "wsum": W0[2:] + res0_W[2:],
        "b0": pbias(b0),
        "eye": np.ascontiguousarray(np.eye(128, dtype=ml_dtypes.bfloat16)),
        "hw1": np.ascontiguousarray(stat(hW1)),
        "hb1": pbias(hb1),
        "hw2": _hw2pad(stat(hW2)),
        "hb2": hb2.reshape(2, 1),
    }
    for i in range(3):
        shared[f"w{i + 1}"] = tobf(stat(Ws[i] / 3.0))
        shared[f"b{i + 1}"] = pbias(bs[i])

    in_maps = []
    for c in range(NCORES):
        sl = slice(c * BPC, (c + 1) * BPC)
        m = dict(shared)
        # (BPC, 2048) -> (BPC, V, 2) -> (BPC, 2, V) -> (2*BPC, V): row 2s+c = x[s, c::2]
        m["x"] = np.ascontiguousarray(
            x[sl].reshape(BPC, V, 2).transpose(0, 2, 1).reshape(2 * BPC, V))
        m["embT"] = np.ascontiguousarray(emb[sl].T)
        in_maps.append(m)
    return in_maps


def kernel(**inputs) -> np.ndarray:
    in_maps = build_in_maps(inputs)
    nc = _get_prog()
    res = run_bass_kernel_spmd(nc, in_maps, list(range(NCORES)))
    outs = []
    for i in range(NCORES):
        o = res.results[i]["out"]  # (2*BPC, V), row 2s+c = out[s, c::2]
        outs.append(o.reshape(BPC, 2, V).transpose(0, 2, 1).reshape(BPC, DATA))
    return np.concatenate(outs, axis=0)


if __name__ == "__main__":
    rng = np.random.default_rng(0)
    demo = {
        "x": rng.standard_normal((B, DATA), dtype=np.float32),
        "t": rng.integers(0, 1000, size=(B,)).astype(np.int32),
        "time_W": rng.standard_normal((TDIM, TDIM), dtype=np.float32) / 11.3,
        "time_b": np.zeros(TDIM, np.float32),
        "W0": rng.standard_normal((130, HID), dtype=np.float32) / 11.4,
        "b0": np.zeros(HID, np.float32),
        "W1": rng.standard_normal((HID, HID), dtype=np.float32) / 16.0,
        "b1": np.zeros(HID, np.float32),
        "W2": rng.standard_normal((HID, HID), dtype=np.float32) / 16.0,
        "b2": np.zeros(HID, np.float32),
        "W3": rng.standard_normal((HID, HID), dtype=np.float32) / 16.0,
        "b3": np.zeros(HID, np.float32),
        "res0_W": rng.standard_normal((130, HID), dtype=np.float32) / 11.4,
        "hW1": rng.standard_normal((HID, HID), dtype=np.float32) / 16.0,
        "hb1": np.zeros(HID, np.float32),
        "hW2": rng.standard_normal((HID, 2), dtype=np.float32) / 16.0,
        "hb2": np.zeros(2, np.float32),
    }
    out = kernel(**demo)
    print("out", out.shape, out.dtype, float(np.abs(out).mean()))

